# revision 14
# baseline (speedup 1.0000x reference)
"""3-layer GAT on 8 Trainium2 NeuronCores (Bass/Tile).

Strategy (dst-sharded, edge phase fully local per core):
- Node dim sharded 8-ways: core c owns dst nodes [c*6250, (c+1)*6250).
- Host sorts edges by dst, groups them into 128-dst "windows" (49/core),
  splits each window's edges by record-row < 32768 (int16 gather index limit),
  and pads each (window, lo/hi) group to a multiple of 128 "slots". Group
  sizes are equalized across cores (max over cores) so one SPMD program
  serves all 8 cores.
- Per layer: a dense phase computes per-node records
  [h bf16 | 1.0 | el f32] (512B each) for the core's slice; an AllGather
  replicates the full 50176-row record table; the edge phase dma_gathers
  records by src (<=1024 rows per call, round-robin over 4 SWDGE queues),
  computes scores e = el[src] + er[dst] (er via one-hot-transpose matmuls
  against per-dst er hi/lo), weights w = exp(leaky_relu(e)) =
  max(exp(e), exp(0.2 e)), scales the gathered records in place by w, and
  aggregates out[d] = sum_e w_e*h[src_e] / sum_e w_e with one-hot EQ
  matmuls on the tensor engine (the record's ones-column supplies the
  denominator). All one-hot matrices are built with block-batched
  tensor_tensor compares on broadcast access patterns (no per-chunk
  AP-scalar tensor_scalar, which is ~10x slower).
- Layer k's edge-phase epilogue computes layer k+1's dense phase in place
  (ACT relu-with-reciprocal-scale, PE transpose, matmul with
  [W | wl_hi | wl_lo | wr_hi | wr_lo]); per-window el/er bookkeeping is
  staged and finalized with per-layer batched ops.
- Final readout: per-core partial sum over owned dst rows, host adds the 8
  partials.
"""
import hashlib
import os
import sys
import time

import numpy as np

sys.path.insert(0, "/opt/trn_rl_repo")
sys.path.insert(0, "/root/.axon_site/_ro/trn_rl_repo")

import ml_dtypes

import concourse.bacc as bacc
import concourse.mybir as mybir
import concourse.tile as tile
from concourse.bass_utils import run_bass_kernel_spmd
from concourse.library_config import mlp as mlp_lib

BF16 = ml_dtypes.bfloat16
F32 = mybir.dt.float32
BF = mybir.dt.bfloat16
I16 = mybir.dt.int16
I8 = mybir.dt.int8
AL = mybir.AluOpType

N_NODES = 50000
N_EDGES = 800000
HID = 128
OUT_DIM = 64
N_CORES = 8
SLICE = N_NODES // N_CORES          # 6250
NW = (SLICE + 127) // 128           # 49 windows per core
SLICE_PAD = NW * 128                # 6272
NROWS = SLICE_PAD * N_CORES         # 50176 record rows
HI_BASE = 32768                     # record rows >= HI_BASE use the offset gather
WPB = 3                             # windows per block
REC = 256                           # record bf16 elems (512B)
GCAP = 1024                         # max idxs per dma_gather call (SWDGE ring)

_CACHE = {}
LAST_RESULTS = None  # set by kernel(); test harness reads exec_time_ns off this


def _row_of_node(n):
    return (n // SLICE) * SLICE_PAD + (n % SLICE)


def _wrap_idxs(idx):
    """[n] int array -> [128, n/16] int16 wrapped-in-16, replicated across cores."""
    n = idx.shape[0]
    assert n % 16 == 0
    w = idx.reshape(n // 16, 16).T
    return np.tile(w, (8, 1)).astype(np.int16)


def _host_prep(src, dst):
    src = np.asarray(src).astype(np.int64)
    dst = np.asarray(dst).astype(np.int64)
    rows = _row_of_node(src)

    per_cw = {}
    n_lo = np.zeros((N_CORES, NW), np.int64)
    n_hi = np.zeros((N_CORES, NW), np.int64)
    for c in range(N_CORES):
        m = (dst >= c * SLICE) & (dst < (c + 1) * SLICE)
        dl = dst[m] - c * SLICE
        rs = rows[m]
        order = np.argsort(dl, kind="stable")
        dl = dl[order]
        rs = rs[order]
        bounds = np.searchsorted(dl, np.arange(0, SLICE_PAD + 1, 128))
        for w in range(NW):
            a, b = bounds[w], bounds[w + 1]
            wdl = dl[a:b]
            wrs = rs[a:b]
            lo = wrs < HI_BASE
            per_cw[(c, w)] = (wdl[lo], wrs[lo], wdl[~lo], wrs[~lo])
            n_lo[c, w] = lo.sum()
            n_hi[c, w] = (~lo).sum()

    L = np.maximum(1, -(-n_lo.max(axis=0) // 128))
    H = np.maximum(1, -(-n_hi.max(axis=0) // 128))

    blocks = [list(range(b, min(b + WPB, NW))) for b in range(0, NW, WPB)]

    meta_blocks = []
    tot_ch = 0
    tot_slots = 0
    lo_cols = 0
    hi_cols = 0
    for wins in blocks:
        nlo_ch = int(sum(L[w] for w in wins))
        nhi_ch = int(sum(H[w] for w in wins))
        nch = nlo_ch + nhi_ch
        ch_w = []
        for w in wins:
            ch_w += [w] * int(L[w])
        for w in wins:
            ch_w += [w] * int(H[w])
        first = {w: ch_w.index(w) for w in wins}
        last = {w: len(ch_w) - 1 - ch_w[::-1].index(w) for w in wins}
        meta_blocks.append(dict(
            wins=wins, nlo_ch=nlo_ch, nhi_ch=nhi_ch, nch=nch, ch_w=ch_w,
            first=first, last=last,
            ch_off=tot_ch, slot_off=tot_slots,
            lo_col_off=lo_cols, hi_col_off=hi_cols,
        ))
        tot_ch += nch
        tot_slots += nch * 128
        lo_cols += nlo_ch * 128 // 16
        hi_cols += nhi_ch * 128 // 16

    per_core = []
    for c in range(N_CORES):
        idx_lo_cols = []
        idx_hi_cols = []
        dstch = np.full((128, tot_ch), -1.0, np.float32)
        dstrow = np.full((1, tot_slots), -1, np.int8)
        for mb in meta_blocks:
            lo_idx = []
            hi_idx = []
            lo_dl = []
            hi_dl = []
            for w in mb["wins"]:
                wdl_lo, wrs_lo, wdl_hi, wrs_hi = per_cw[(c, w)]
                npad = int(L[w]) * 128
                pad_i = np.zeros(npad, np.int64)
                pad_d = np.full(npad, -1.0, np.float64)
                pad_i[: len(wrs_lo)] = wrs_lo
                pad_d[: len(wdl_lo)] = wdl_lo - w * 128
                lo_idx.append(pad_i)
                lo_dl.append(pad_d)
                npad = int(H[w]) * 128
                pad_i = np.zeros(npad, np.int64)
                pad_d = np.full(npad, -1.0, np.float64)
                pad_i[: len(wrs_hi)] = wrs_hi - HI_BASE
                pad_d[: len(wdl_hi)] = wdl_hi - w * 128
                hi_idx.append(pad_i)
                hi_dl.append(pad_d)
            lo_idx = np.concatenate(lo_idx)
            hi_idx = np.concatenate(hi_idx)
            dl_all = np.concatenate(lo_dl + hi_dl)
            idx_lo_cols.append(_wrap_idxs(lo_idx))
            idx_hi_cols.append(_wrap_idxs(hi_idx))
            nch = mb["nch"]
            dstch[:, mb["ch_off"]: mb["ch_off"] + nch] = (
                dl_all.reshape(nch, 128).T.astype(np.float32))
            dstrow[0, mb["slot_off"]: mb["slot_off"] + nch * 128] = (
                dl_all.astype(np.int8))
        per_core.append(dict(
            idx_lo=np.concatenate(idx_lo_cols, axis=1),
            idx_hi=np.concatenate(idx_hi_cols, axis=1),
            dstch=dstch.astype(BF16),
            dstrow=dstrow,
        ))

    return dict(
        blocks=meta_blocks, L=L, H=H, per_core=per_core,
        tot_ch=tot_ch, tot_slots=tot_slots,
        lo_cols=lo_cols, hi_cols=hi_cols,
        chmax=max(mb["nch"] for mb in meta_blocks),
    )


def _build_program(meta):
    nc = bacc.Bacc("TRN2", target_bir_lowering=False, debug=False,
                   num_devices=N_CORES, num_swdge_queues=4)

    xT = nc.dram_tensor("xT", [128, SLICE_PAD], F32, kind="ExternalInput")
    w1ext = nc.dram_tensor("w1ext", [128, 132], F32, kind="ExternalInput")
    w2cat = nc.dram_tensor("w2cat", [128, 132], BF, kind="ExternalInput")
    w3cat = nc.dram_tensor("w3cat", [128, 68], BF, kind="ExternalInput")
    idxlo_d = nc.dram_tensor("idxlo", [128, meta["lo_cols"]], I16, kind="ExternalInput")
    idxhi_d = nc.dram_tensor("idxhi", [128, meta["hi_cols"]], I16, kind="ExternalInput")
    dstch_d = nc.dram_tensor("dstch", [128, meta["tot_ch"]], BF, kind="ExternalInput")
    dstrow_d = nc.dram_tensor("dstrow", [1, meta["tot_slots"]], I8, kind="ExternalInput")
    iota_row_d = nc.dram_tensor("iota_row", [128, 128], BF, kind="ExternalInput")
    iota_col8_d = nc.dram_tensor("iota_col8", [128, 1], I8, kind="ExternalInput")
    eye_d = nc.dram_tensor("eye", [128, 128], BF, kind="ExternalInput")
    out_d = nc.dram_tensor("out", [1, OUT_DIM], F32, kind="ExternalOutput")

    # edge-phase params per layer: (h dim in record, el bf16-col)
    EDGE = {1: (HID, 130), 2: (HID, 130), 3: (OUT_DIM, 66)}
    blocks = meta["blocks"]
    CH = meta["chmax"]

    _nlayers = int(os.environ.get("GAT_LAYERS", "3"))
    _nblocks = int(os.environ.get("GAT_BLOCKS", "999"))

    with tile.TileContext(nc) as tc:
        with (
            tc.tile_pool(name="resid", bufs=1) as rp,
            tc.tile_pool(name="work", bufs=2) as wp,
            tc.tile_pool(name="eqp", bufs=2) as eqp,
            tc.tile_pool(name="recs", bufs=3) as rcp,
            tc.tile_pool(name="dram", bufs=1, space="DRAM") as dp,
            tc.tile_pool(name="agg_ps", bufs=4, space="PSUM") as agg_pool,
            tc.tile_pool(name="er_ps", bufs=2, space="PSUM") as er_pool,
            tc.tile_pool(name="dn_ps", bufs=1, space="PSUM") as dn_pool,
            tc.tile_pool(name="tr_ps", bufs=1, space="PSUM") as tr_pool,
        ):
            nc.gpsimd.load_library(mlp_lib)

            # ---- resident tiles ----
            t_xT = rp.tile([128, SLICE_PAD], F32, tag="xT")
            nc.sync.dma_start(out=t_xT[:], in_=xT[:])
            t_w1 = rp.tile([128, 132], F32, tag="w1")
            nc.sync.dma_start(out=t_w1[:], in_=w1ext[:])
            t_w2 = rp.tile([128, 132], BF, tag="w2")
            nc.sync.dma_start(out=t_w2[:], in_=w2cat[:])
            t_w3 = rp.tile([128, 68], BF, tag="w3")
            nc.sync.dma_start(out=t_w3[:], in_=w3cat[:])
            t_idxlo = rp.tile([128, meta["lo_cols"]], I16, tag="idxlo")
            nc.sync.dma_start(out=t_idxlo[:], in_=idxlo_d[:])
            t_idxhi = rp.tile([128, meta["hi_cols"]], I16, tag="idxhi")
            nc.sync.dma_start(out=t_idxhi[:], in_=idxhi_d[:])
            t_dstch = rp.tile([128, meta["tot_ch"]], BF, tag="dstch")
            nc.sync.dma_start(out=t_dstch[:], in_=dstch_d[:])
            t_iota_row = rp.tile([128, 128], BF, tag="iota_row")
            nc.sync.dma_start(out=t_iota_row[:], in_=iota_row_d[:])
            t_iota8 = rp.tile([128, 1], I8, tag="iota8")
            nc.sync.dma_start(out=t_iota8[:], in_=iota_col8_d[:])
            t_eye = rp.tile([128, 128], BF, tag="eye")
            nc.sync.dma_start(out=t_eye[:], in_=eye_d[:])

            t_er = {k: rp.tile([128, NW, 2], BF, tag=f"er{k}", name=f"er{k}")
                    for k in (1, 2, 3)}
            # per-produced-layer staging of [el(2) | er(2)] psD cols
            t_stage = {k: rp.tile([128, NW, 4], F32, tag=f"st{k}", name=f"st{k}")
                       for k in (1, 2, 3)}
            t_acc = rp.tile([128, OUT_DIM], F32, tag="acc")
            nc.vector.memset(t_acc[:], 0.0)
            t_ones_col = rp.tile([128, 1], F32, tag="ones_col")
            nc.vector.memset(t_ones_col[:], 1.0)
            ones_bf = nc.const_aps.tensor(1.0, (128, 1), BF)

            rec_slices = {k: dp.tile([SLICE_PAD, REC], BF, name=f"rec_slice{k}")
                          for k in (1, 2, 3)}
            rec_fulls = {k: dp.tile([NROWS, REC], BF, addr_space="Shared",
                                    name=f"rec_full{k}")
                         for k in (1, 2, 3)}

            def epilogue_dense(kprod, w, psD):
                """Per-window: record h+ones assembly + stage el/er cols."""
                hdim = HID if kprod < 3 else OUT_DIM
                rec_t = rcp.tile([128, REC], BF, tag="rec_out", name="rec_out")
                nc.vector.memset(rec_t[:], 0.0)
                nc.scalar.copy(rec_t[:, 0:hdim], psD[:, 0:hdim])
                nc.vector.tensor_copy(rec_t[:, hdim:hdim + 1], ones_bf)
                if kprod == 1:
                    nc.scalar.copy(t_stage[1][:, w, 0:2], psD[:, 128:130])
                else:
                    nc.scalar.copy(t_stage[kprod][:, w, :], psD[:, hdim:hdim + 4])
                nc.sync.dma_start(
                    out=rec_slices[kprod][w * 128:(w + 1) * 128, :], in_=rec_t[:])

            def finalize_dense(kprod):
                """Per-layer batched: el -> record table, er -> hi/lo."""
                hdim = HID if kprod < 3 else OUT_DIM
                elcol = hdim + 2
                st = t_stage[kprod]
                elv = wp.tile([128, NW], F32, tag="elv", name="elv")
                erv = wp.tile([128, NW], F32, tag="erv", name="erv")
                if kprod == 1:
                    nc.vector.tensor_copy(elv[:], st[:, :, 0])
                    nc.vector.tensor_copy(erv[:], st[:, :, 1])
                else:
                    nc.vector.tensor_tensor(elv[:], st[:, :, 0], st[:, :, 1], AL.add)
                    nc.vector.tensor_tensor(erv[:], st[:, :, 2], st[:, :, 3], AL.add)
                # el into the record table's el field (strided 4B writes)
                el_dst = (rec_slices[kprod]
                          .rearrange("(w p) j -> p w j", p=128)[:, :, elcol:elcol + 2]
                          .bitcast(F32))
                nc.sync.dma_start(out=el_dst, in_=elv[:, :, None])
                # er hi/lo
                erh = t_er[kprod][:, :, 0:1]
                erl = t_er[kprod][:, :, 1:2]
                nc.vector.tensor_copy(erh, erv[:, :, None])
                t32 = wp.tile([128, NW], F32, tag="er32", name="er32")
                nc.vector.tensor_copy(t32[:], erh)
                nc.vector.tensor_tensor(t32[:], erv[:], t32[:], AL.subtract)
                nc.vector.tensor_copy(erl, t32[:, :, None])

            def dense_from_x(kprod, w, xq_bf):
                ps_t = tr_pool.tile([128, 128], BF, tag="tr", name="tr")
                nc.tensor.transpose(ps_t[:], xq_bf, t_eye[:])
                xTt = wp.tile([128, 128], BF, tag="xTt", name="xTt")
                nc.scalar.copy(xTt[:], ps_t[:])
                wcat = t_w2 if kprod == 2 else t_w3
                ncols = 132 if kprod == 2 else 68
                psD = dn_pool.tile([128, 132], F32, tag="dn", name="dn")
                nc.tensor.matmul(psD[:, 0:ncols], xTt[:], wcat[:, 0:ncols],
                                 start=True, stop=True)
                epilogue_dense(kprod, w, psD)

            # ================= Layer 1 dense =================
            for w in range(NW):
                psD = dn_pool.tile([128, 132], F32, tag="dn", name="dn")
                nc.tensor.matmul(psD[:], t_xT[:, w * 128:(w + 1) * 128],
                                 t_w1[:], start=True, stop=True)
                epilogue_dense(1, w, psD)
            finalize_dense(1)

            # ================= 3 GAT layers =================
            qrr = [0]  # gather queue round-robin

            for k in (1, 2, 3)[:_nlayers]:
                hdim, elcol = EDGE[k]
                rec_full = rec_fulls[k]
                nc.gpsimd.collective_compute(
                    "AllGather", AL.bypass,
                    replica_groups=[list(range(N_CORES))],
                    ins=[rec_slices[k].opt()], outs=[rec_full.opt()],
                )
                for mb in blocks[:_nblocks]:
                    nch = mb["nch"]
                    nlo = mb["nlo_ch"] * 128
                    nhi = mb["nhi_ch"] * 128
                    rec = wp.tile([128, CH, REC], BF, tag="rec", name="rec")
                    for g0 in range(0, nlo, GCAP):
                        gn = min(GCAP, nlo - g0)
                        nc.gpsimd.dma_gather(
                            rec[:, g0 // 128:(g0 + gn) // 128, :],
                            rec_full[0:HI_BASE, :],
                            t_idxlo[:, mb["lo_col_off"] + g0 // 16:
                                    mb["lo_col_off"] + (g0 + gn) // 16],
                            gn, gn, REC, queue_num=qrr[0] % 4)
                        qrr[0] += 1
                    for g0 in range(0, nhi, GCAP):
                        gn = min(GCAP, nhi - g0)
                        nc.gpsimd.dma_gather(
                            rec[:, mb["nlo_ch"] + g0 // 128:
                                mb["nlo_ch"] + (g0 + gn) // 128, :],
                            rec_full[HI_BASE:NROWS, :],
                            t_idxhi[:, mb["hi_col_off"] + g0 // 16:
                                    mb["hi_col_off"] + (g0 + gn) // 16],
                            gn, gn, REC, queue_num=qrr[0] % 4)
                        qrr[0] += 1
                    # dst_local replicated across partitions (int8, DRAM bcast)
                    rep = wp.tile([128, CH, 128], I8, tag="rep", name="rep")
                    nc.sync.dma_start(
                        out=rep[:, 0:nch, :],
                        in_=dstrow_d[0:1, mb["slot_off"]: mb["slot_off"] + nch * 128]
                        .to_broadcast((128, nch * 128)))
                    # STr[d, ci, e] = (dst_local[slot] == d), batched
                    STrt = eqp.tile([128, CH, 128], BF, tag="STr", name="STr")
                    nc.vector.tensor_tensor(
                        STrt[:, 0:nch, :], rep[:, 0:nch, :],
                        t_iota8[:, :, None].to_broadcast((128, nch, 128)),
                        AL.is_equal)
                    # er expansion via STr matmuls (N=2 hi/lo)
                    ps_er = er_pool.tile([128, 2 * CH], F32, tag="er", name="er")
                    for ci in range(nch):
                        nc.tensor.matmul(
                            ps_er[:, 2 * ci:2 * ci + 2], STrt[:, ci, :],
                            t_er[k][:, mb["ch_w"][ci], :],
                            start=True, stop=True)
                    er2 = wp.tile([128, 2 * CH], F32, tag="er2", name="er2")
                    nc.vector.tensor_copy(er2[:, 0:2 * nch], ps_er[:, 0:2 * nch])
                    erc = wp.tile([128, CH], F32, tag="erc", name="erc")
                    nc.vector.tensor_tensor(
                        erc[:, 0:nch], er2[:, 0:2 * nch:2], er2[:, 1:2 * nch:2],
                        AL.add)
                    # el + scores + weights (compact)
                    elc = wp.tile([128, CH], F32, tag="elc", name="elc")
                    nc.vector.tensor_copy(
                        elc[:, 0:nch],
                        rec[:, 0:nch, elcol:elcol + 2].bitcast(F32))
                    ec = wp.tile([128, CH], F32, tag="ec", name="ec")
                    nc.vector.tensor_tensor(ec[:, 0:nch], elc[:, 0:nch],
                                            erc[:, 0:nch], AL.add)
                    ex1 = wp.tile([128, CH], BF, tag="ex1", name="ex1")
                    nc.scalar.activation(ex1[:, 0:nch], ec[:, 0:nch],
                                         mybir.ActivationFunctionType.Exp,
                                         bias=0.0, scale=1.0)
                    ex2 = wp.tile([128, CH], BF, tag="ex2", name="ex2")
                    nc.scalar.activation(ex2[:, 0:nch], ec[:, 0:nch],
                                         mybir.ActivationFunctionType.Exp,
                                         bias=0.0, scale=0.2)
                    exc = wp.tile([128, CH], BF, tag="exc", name="exc")
                    nc.vector.tensor_tensor(exc[:, 0:nch], ex1[:, 0:nch],
                                            ex2[:, 0:nch], AL.max)
                    # scale gathered records (h + ones cols) in place by w_e
                    nc.vector.tensor_tensor(
                        rec[:, 0:nch, 0:hdim + 1], rec[:, 0:nch, 0:hdim + 1],
                        exc[:, 0:nch, None].to_broadcast((128, nch, hdim + 1)),
                        AL.mult)
                    # EQ[e, ci, d] = (dst_local[slot] == d), batched
                    EQt = eqp.tile([128, CH, 128], BF, tag="EQ", name="EQ")
                    nc.vector.tensor_tensor(
                        EQt[:, 0:nch, :],
                        t_iota_row[:, None, :].to_broadcast((128, nch, 128)),
                        t_dstch[:, mb["ch_off"]:mb["ch_off"] + nch][:, :, None]
                        .to_broadcast((128, nch, 128)),
                        AL.is_equal)
                    # aggregation matmuls
                    psA = {w: agg_pool.tile([128, 132], F32, tag="agg",
                                            name=f"agg{w}")
                           for w in mb["wins"]}
                    for ci in range(nch):
                        w = mb["ch_w"][ci]
                        nc.tensor.matmul(
                            psA[w][:, 0:hdim + 1], EQt[:, ci, :],
                            rec[:, ci, 0:hdim + 1],
                            start=(ci == mb["first"][w]),
                            stop=(ci == mb["last"][w]))
                    # window epilogues
                    for w in mb["wins"]:
                        s_eps = wp.tile([128, 1], F32, tag="s_eps", name="s_eps")
                        nc.vector.tensor_scalar(
                            s_eps[:], psA[w][:, hdim:hdim + 1], 1e-30, None,
                            AL.add)
                        rcpv = wp.tile([128, 1], F32, tag="rcp", name="rcp")
                        nc.vector.reciprocal(rcpv[:], s_eps[:])
                        if k < 3:
                            xq = rcp.tile([128, 128], BF, tag="xq", name="xq")
                            nc.scalar.activation(
                                xq[:], psA[w][:, 0:hdim],
                                mybir.ActivationFunctionType.Relu,
                                bias=0.0, scale=rcpv[:])
                            dense_from_x(k + 1, w, xq[:])
                        else:
                            t3 = wp.tile([128, OUT_DIM], F32, tag="t3", name="t3")
                            nc.scalar.activation(
                                t3[:], psA[w][:, 0:OUT_DIM],
                                mybir.ActivationFunctionType.Relu,
                                bias=0.0, scale=rcpv[:])
                            nc.vector.tensor_tensor(
                                t_acc[:], t_acc[:], t3[:], AL.add)
                if k < 3:
                    finalize_dense(k + 1)

            # ================= readout =================
            psR = dn_pool.tile([128, 132], F32, tag="dn", name="dnR")
            nc.tensor.matmul(psR[0:1, 0:OUT_DIM], t_ones_col[:], t_acc[:],
                             start=True, stop=True)
            t_out = wp.tile([1, OUT_DIM], F32, tag="outv", name="outv")
            nc.vector.tensor_copy(t_out[:], psR[0:1, 0:OUT_DIM])
            nc.sync.dma_start(out=out_d[:], in_=t_out[:])

    nc.compile()
    return nc


def _hilo(v):
    hi = v.astype(BF16)
    lo = (v - hi.astype(np.float32)).astype(BF16)
    return hi, lo


def kernel(**inputs):
    global LAST_RESULTS
    x = np.ascontiguousarray(np.asarray(inputs["x"], np.float32))
    src = np.asarray(inputs["src"])
    dst = np.asarray(inputs["dst"])
    W1 = np.asarray(inputs["W1"], np.float32)
    al1 = np.asarray(inputs["al1"], np.float32)
    ar1 = np.asarray(inputs["ar1"], np.float32)
    W2 = np.asarray(inputs["W2"], np.float32)
    al2 = np.asarray(inputs["al2"], np.float32)
    ar2 = np.asarray(inputs["ar2"], np.float32)
    W3 = np.asarray(inputs["W3"], np.float32)
    al3 = np.asarray(inputs["al3"], np.float32)
    ar3 = np.asarray(inputs["ar3"], np.float32)

    key = hashlib.sha1(src.tobytes() + dst.tobytes()).hexdigest()
    if key not in _CACHE:
        t0 = time.time()
        meta = _host_prep(src, dst)
        t1 = time.time()
        nc = _build_program(meta)
        print(f"[kernel] host prep {t1 - t0:.1f}s, build+compile "
              f"{time.time() - t1:.1f}s", file=sys.stderr)
        _CACHE[key] = (meta, nc)
    meta, nc = _CACHE[key]

    w1ext = np.zeros((128, 132), np.float32)
    w1ext[:, 0:128] = W1
    w1ext[:, 128] = W1 @ al1
    w1ext[:, 129] = W1 @ ar1
    w2cat = np.zeros((128, 132), BF16)
    w2cat[:, 0:128] = W2.astype(BF16)
    w2cat[:, 128], w2cat[:, 129] = _hilo(W2 @ al2)
    w2cat[:, 130], w2cat[:, 131] = _hilo(W2 @ ar2)
    w3cat = np.zeros((128, 68), BF16)
    w3cat[:, 0:64] = W3.astype(BF16)
    w3cat[:, 64], w3cat[:, 65] = _hilo(W3 @ al3)
    w3cat[:, 66], w3cat[:, 67] = _hilo(W3 @ ar3)
    iota_row = np.tile(np.arange(128, dtype=np.float32).astype(BF16)[None, :],
                       (128, 1))
    iota_col8 = np.arange(128, dtype=np.int8).reshape(128, 1)
    eye = np.eye(128, dtype=np.float32).astype(BF16)

    in_maps = []
    for c in range(N_CORES):
        xs = np.zeros((SLICE_PAD, 128), np.float32)
        xs[0:SLICE] = x[c * SLICE:(c + 1) * SLICE]
        pc = meta["per_core"][c]
        in_maps.append(dict(
            xT=np.ascontiguousarray(xs.T),
            w1ext=w1ext, w2cat=w2cat, w3cat=w3cat,
            idxlo=pc["idx_lo"], idxhi=pc["idx_hi"],
            dstch=pc["dstch"], dstrow=pc["dstrow"],
            iota_row=iota_row, iota_col8=iota_col8, eye=eye,
        ))

    res = run_bass_kernel_spmd(nc, in_maps, core_ids=list(range(N_CORES)))
    LAST_RESULTS = res
    out = np.zeros(OUT_DIM, np.float32)
    for c in range(N_CORES):
        out += np.asarray(res.results[c]["out"], np.float32)[0]
    return out


# revision 15
# speedup vs baseline: 1.0162x; 1.0162x over previous
"""3-layer GAT on 8 Trainium2 NeuronCores (Bass/Tile).

Strategy (dst-sharded, edge phase fully local per core):
- Node dim sharded 8-ways: core c owns dst nodes [c*6250, (c+1)*6250).
- Host sorts edges by dst, groups them into 128-dst "windows" (49/core),
  splits each window's edges by record-row < 32768 (int16 gather index limit),
  and pads each (window, lo/hi) group to a multiple of 128 "slots". Group
  sizes are equalized across cores (max over cores) so one SPMD program
  serves all 8 cores.
- Per layer: a dense phase computes per-node records
  [h bf16 | 1.0 | el f32] (512B each) for the core's slice; an AllGather
  replicates the full 50176-row record table; the edge phase dma_gathers
  records by src (<=1024 rows per call, round-robin over 4 SWDGE queues),
  computes scores e = el[src] + er[dst] (er via one-hot-transpose matmuls
  against per-dst er hi/lo), weights w = exp(leaky_relu(e)) =
  max(exp(e), exp(0.2 e)), scales the gathered records in place by w, and
  aggregates out[d] = sum_e w_e*h[src_e] / sum_e w_e with one-hot EQ
  matmuls on the tensor engine (the record's ones-column supplies the
  denominator). All one-hot matrices are built with block-batched
  tensor_tensor compares on broadcast access patterns (no per-chunk
  AP-scalar tensor_scalar, which is ~10x slower).
- Layer k's edge-phase epilogue computes layer k+1's dense phase in place
  (ACT relu-with-reciprocal-scale, PE transpose, matmul with
  [W | wl_hi | wl_lo | wr_hi | wr_lo]); per-window el/er bookkeeping is
  staged and finalized with per-layer batched ops.
- Final readout: per-core partial sum over owned dst rows, host adds the 8
  partials.
"""
import hashlib
import os
import sys
import time

import numpy as np

sys.path.insert(0, "/opt/trn_rl_repo")
sys.path.insert(0, "/root/.axon_site/_ro/trn_rl_repo")

import ml_dtypes

import concourse.bacc as bacc
import concourse.mybir as mybir
import concourse.tile as tile
from concourse.bass_utils import run_bass_kernel_spmd
from concourse.library_config import mlp as mlp_lib

BF16 = ml_dtypes.bfloat16
F32 = mybir.dt.float32
BF = mybir.dt.bfloat16
I16 = mybir.dt.int16
I8 = mybir.dt.int8
AL = mybir.AluOpType

N_NODES = 50000
N_EDGES = 800000
HID = 128
OUT_DIM = 64
N_CORES = 8
SLICE = N_NODES // N_CORES          # 6250
NW = (SLICE + 127) // 128           # 49 windows per core
SLICE_PAD = NW * 128                # 6272
NROWS = SLICE_PAD * N_CORES         # 50176 record rows
HI_BASE = 32768                     # record rows >= HI_BASE use the offset gather
WPB = 3                             # windows per block
REC = 256                           # record bf16 elems (512B)
GCAP = 1024                         # max idxs per dma_gather call (SWDGE ring)

_CACHE = {}
LAST_RESULTS = None  # set by kernel(); test harness reads exec_time_ns off this


def _row_of_node(n):
    return (n // SLICE) * SLICE_PAD + (n % SLICE)


def _wrap_idxs(idx):
    """[n] int array -> [128, n/16] int16 wrapped-in-16, replicated across cores."""
    n = idx.shape[0]
    assert n % 16 == 0
    w = idx.reshape(n // 16, 16).T
    return np.tile(w, (8, 1)).astype(np.int16)


def _host_prep(src, dst):
    src = np.asarray(src).astype(np.int64)
    dst = np.asarray(dst).astype(np.int64)
    rows = _row_of_node(src)

    per_cw = {}
    n_lo = np.zeros((N_CORES, NW), np.int64)
    n_hi = np.zeros((N_CORES, NW), np.int64)
    for c in range(N_CORES):
        m = (dst >= c * SLICE) & (dst < (c + 1) * SLICE)
        dl = dst[m] - c * SLICE
        rs = rows[m]
        order = np.argsort(dl, kind="stable")
        dl = dl[order]
        rs = rs[order]
        bounds = np.searchsorted(dl, np.arange(0, SLICE_PAD + 1, 128))
        for w in range(NW):
            a, b = bounds[w], bounds[w + 1]
            wdl = dl[a:b]
            wrs = rs[a:b]
            lo = wrs < HI_BASE
            per_cw[(c, w)] = (wdl[lo], wrs[lo], wdl[~lo], wrs[~lo])
            n_lo[c, w] = lo.sum()
            n_hi[c, w] = (~lo).sum()

    L = np.maximum(1, -(-n_lo.max(axis=0) // 128))
    H = np.maximum(1, -(-n_hi.max(axis=0) // 128))

    blocks = [list(range(b, min(b + WPB, NW))) for b in range(0, NW, WPB)]

    meta_blocks = []
    tot_ch = 0
    tot_slots = 0
    lo_cols = 0
    hi_cols = 0
    for wins in blocks:
        nlo_ch = int(sum(L[w] for w in wins))
        nhi_ch = int(sum(H[w] for w in wins))
        nch = nlo_ch + nhi_ch
        ch_w = []
        for w in wins:
            ch_w += [w] * int(L[w])
        for w in wins:
            ch_w += [w] * int(H[w])
        first = {w: ch_w.index(w) for w in wins}
        last = {w: len(ch_w) - 1 - ch_w[::-1].index(w) for w in wins}
        meta_blocks.append(dict(
            wins=wins, nlo_ch=nlo_ch, nhi_ch=nhi_ch, nch=nch, ch_w=ch_w,
            first=first, last=last,
            ch_off=tot_ch, slot_off=tot_slots,
            lo_col_off=lo_cols, hi_col_off=hi_cols,
        ))
        tot_ch += nch
        tot_slots += nch * 128
        lo_cols += nlo_ch * 128 // 16
        hi_cols += nhi_ch * 128 // 16

    per_core = []
    for c in range(N_CORES):
        idx_lo_cols = []
        idx_hi_cols = []
        dstch = np.full((128, tot_ch), -1.0, np.float32)
        dstrow = np.full((1, tot_slots), -1, np.int8)
        for mb in meta_blocks:
            lo_idx = []
            hi_idx = []
            lo_dl = []
            hi_dl = []
            for w in mb["wins"]:
                wdl_lo, wrs_lo, wdl_hi, wrs_hi = per_cw[(c, w)]
                npad = int(L[w]) * 128
                pad_i = np.zeros(npad, np.int64)
                pad_d = np.full(npad, -1.0, np.float64)
                pad_i[: len(wrs_lo)] = wrs_lo
                pad_d[: len(wdl_lo)] = wdl_lo - w * 128
                lo_idx.append(pad_i)
                lo_dl.append(pad_d)
                npad = int(H[w]) * 128
                pad_i = np.zeros(npad, np.int64)
                pad_d = np.full(npad, -1.0, np.float64)
                pad_i[: len(wrs_hi)] = wrs_hi - HI_BASE
                pad_d[: len(wdl_hi)] = wdl_hi - w * 128
                hi_idx.append(pad_i)
                hi_dl.append(pad_d)
            lo_idx = np.concatenate(lo_idx)
            hi_idx = np.concatenate(hi_idx)
            dl_all = np.concatenate(lo_dl + hi_dl)
            idx_lo_cols.append(_wrap_idxs(lo_idx))
            idx_hi_cols.append(_wrap_idxs(hi_idx))
            nch = mb["nch"]
            dstch[:, mb["ch_off"]: mb["ch_off"] + nch] = (
                dl_all.reshape(nch, 128).T.astype(np.float32))
            dstrow[0, mb["slot_off"]: mb["slot_off"] + nch * 128] = (
                dl_all.astype(np.int8))
        per_core.append(dict(
            idx_lo=np.concatenate(idx_lo_cols, axis=1),
            idx_hi=np.concatenate(idx_hi_cols, axis=1),
            dstch=dstch.astype(BF16),
            dstrow=dstrow,
        ))

    return dict(
        blocks=meta_blocks, L=L, H=H, per_core=per_core,
        tot_ch=tot_ch, tot_slots=tot_slots,
        lo_cols=lo_cols, hi_cols=hi_cols,
        chmax=max(mb["nch"] for mb in meta_blocks),
    )


def _build_program(meta):
    nc = bacc.Bacc("TRN2", target_bir_lowering=False, debug=False,
                   num_devices=N_CORES, num_swdge_queues=4)

    xT = nc.dram_tensor("xT", [128, SLICE_PAD], F32, kind="ExternalInput")
    w1ext = nc.dram_tensor("w1ext", [128, 132], F32, kind="ExternalInput")
    w2cat = nc.dram_tensor("w2cat", [128, 132], BF, kind="ExternalInput")
    w3cat = nc.dram_tensor("w3cat", [128, 68], BF, kind="ExternalInput")
    idxlo_d = nc.dram_tensor("idxlo", [128, meta["lo_cols"]], I16, kind="ExternalInput")
    idxhi_d = nc.dram_tensor("idxhi", [128, meta["hi_cols"]], I16, kind="ExternalInput")
    dstch_d = nc.dram_tensor("dstch", [128, meta["tot_ch"]], BF, kind="ExternalInput")
    dstrow_d = nc.dram_tensor("dstrow", [1, meta["tot_slots"]], I8, kind="ExternalInput")
    iota_row_d = nc.dram_tensor("iota_row", [128, 128], BF, kind="ExternalInput")
    iota_col8_d = nc.dram_tensor("iota_col8", [128, 1], I8, kind="ExternalInput")
    eye_d = nc.dram_tensor("eye", [128, 128], BF, kind="ExternalInput")
    out_d = nc.dram_tensor("out", [1, OUT_DIM], F32, kind="ExternalOutput")

    # edge-phase params per layer: (h dim in record, el bf16-col)
    EDGE = {1: (HID, 130), 2: (HID, 130), 3: (OUT_DIM, 66)}
    blocks = meta["blocks"]
    CH = meta["chmax"]

    _nlayers = int(os.environ.get("GAT_LAYERS", "3"))
    _nblocks = int(os.environ.get("GAT_BLOCKS", "999"))

    with tile.TileContext(nc) as tc:
        with (
            tc.tile_pool(name="resid", bufs=1) as rp,
            tc.tile_pool(name="work", bufs=2) as wp,
            tc.tile_pool(name="eqp", bufs=2) as eqp,
            tc.tile_pool(name="recs", bufs=3) as rcp,
            tc.tile_pool(name="dram", bufs=1, space="DRAM") as dp,
            tc.tile_pool(name="agg_ps", bufs=5, space="PSUM") as agg_pool,
            tc.tile_pool(name="er_ps", bufs=1, space="PSUM") as er_pool,
            tc.tile_pool(name="dn_ps", bufs=1, space="PSUM") as dn_pool,
            tc.tile_pool(name="tr_ps", bufs=1, space="PSUM") as tr_pool,
        ):
            nc.gpsimd.load_library(mlp_lib)

            # ---- resident tiles ----
            t_xT = rp.tile([128, SLICE_PAD], F32, tag="xT")
            nc.sync.dma_start(out=t_xT[:], in_=xT[:])
            t_w1 = rp.tile([128, 132], F32, tag="w1")
            nc.sync.dma_start(out=t_w1[:], in_=w1ext[:])
            t_w2 = rp.tile([128, 132], BF, tag="w2")
            nc.sync.dma_start(out=t_w2[:], in_=w2cat[:])
            t_w3 = rp.tile([128, 68], BF, tag="w3")
            nc.sync.dma_start(out=t_w3[:], in_=w3cat[:])
            t_idxlo = rp.tile([128, meta["lo_cols"]], I16, tag="idxlo")
            nc.sync.dma_start(out=t_idxlo[:], in_=idxlo_d[:])
            t_idxhi = rp.tile([128, meta["hi_cols"]], I16, tag="idxhi")
            nc.sync.dma_start(out=t_idxhi[:], in_=idxhi_d[:])
            t_dstch = rp.tile([128, meta["tot_ch"]], BF, tag="dstch")
            nc.sync.dma_start(out=t_dstch[:], in_=dstch_d[:])
            t_iota_row = rp.tile([128, 128], BF, tag="iota_row")
            nc.sync.dma_start(out=t_iota_row[:], in_=iota_row_d[:])
            t_iota8 = rp.tile([128, 1], I8, tag="iota8")
            nc.sync.dma_start(out=t_iota8[:], in_=iota_col8_d[:])
            t_eye = rp.tile([128, 128], BF, tag="eye")
            nc.sync.dma_start(out=t_eye[:], in_=eye_d[:])

            t_er = {k: rp.tile([128, NW, 2], BF, tag=f"er{k}", name=f"er{k}")
                    for k in (1, 2, 3)}
            # per-produced-layer staging of [el(2) | er(2)] psD cols
            t_stage = {k: rp.tile([128, NW, 4], F32, tag=f"st{k}", name=f"st{k}")
                       for k in (1, 2, 3)}
            t_acc = rp.tile([128, OUT_DIM], F32, tag="acc")
            nc.vector.memset(t_acc[:], 0.0)
            t_ones_col = rp.tile([128, 1], F32, tag="ones_col")
            nc.vector.memset(t_ones_col[:], 1.0)
            ones_bf = nc.const_aps.tensor(1.0, (128, 1), BF)

            rec_slices = {k: dp.tile([SLICE_PAD, REC], BF, name=f"rec_slice{k}")
                          for k in (1, 2, 3)}
            rec_fulls = {k: dp.tile([NROWS, REC], BF, addr_space="Shared",
                                    name=f"rec_full{k}")
                         for k in (1, 2, 3)}

            def epilogue_dense(kprod, w, psD):
                """Per-window: record h+ones assembly + stage el/er cols."""
                hdim = HID if kprod < 3 else OUT_DIM
                rec_t = rcp.tile([128, REC], BF, tag="rec_out", name="rec_out")
                nc.vector.memset(rec_t[:], 0.0)
                nc.scalar.copy(rec_t[:, 0:hdim], psD[:, 0:hdim])
                nc.vector.tensor_copy(rec_t[:, hdim:hdim + 1], ones_bf)
                if kprod == 1:
                    nc.scalar.copy(t_stage[1][:, w, 0:2], psD[:, 128:130])
                else:
                    nc.scalar.copy(t_stage[kprod][:, w, :], psD[:, hdim:hdim + 4])
                nc.sync.dma_start(
                    out=rec_slices[kprod][w * 128:(w + 1) * 128, :], in_=rec_t[:])

            def finalize_dense(kprod):
                """Per-layer batched: el -> record table, er -> hi/lo."""
                hdim = HID if kprod < 3 else OUT_DIM
                elcol = hdim + 2
                st = t_stage[kprod]
                elv = wp.tile([128, NW], F32, tag="elv", name="elv")
                erv = wp.tile([128, NW], F32, tag="erv", name="erv")
                if kprod == 1:
                    nc.vector.tensor_copy(elv[:], st[:, :, 0])
                    nc.vector.tensor_copy(erv[:], st[:, :, 1])
                else:
                    nc.vector.tensor_tensor(elv[:], st[:, :, 0], st[:, :, 1], AL.add)
                    nc.vector.tensor_tensor(erv[:], st[:, :, 2], st[:, :, 3], AL.add)
                # el into the record table's el field (strided 4B writes)
                el_dst = (rec_slices[kprod]
                          .rearrange("(w p) j -> p w j", p=128)[:, :, elcol:elcol + 2]
                          .bitcast(F32))
                nc.sync.dma_start(out=el_dst, in_=elv[:, :, None])
                # er hi/lo
                erh = t_er[kprod][:, :, 0:1]
                erl = t_er[kprod][:, :, 1:2]
                nc.vector.tensor_copy(erh, erv[:, :, None])
                t32 = wp.tile([128, NW], F32, tag="er32", name="er32")
                nc.vector.tensor_copy(t32[:], erh)
                nc.vector.tensor_tensor(t32[:], erv[:], t32[:], AL.subtract)
                nc.vector.tensor_copy(erl, t32[:, :, None])

            def dense_from_x(kprod, w, xq_bf):
                ps_t = tr_pool.tile([128, 128], BF, tag="tr", name="tr")
                nc.tensor.transpose(ps_t[:], xq_bf, t_eye[:])
                xTt = wp.tile([128, 128], BF, tag="xTt", name="xTt")
                nc.scalar.copy(xTt[:], ps_t[:])
                wcat = t_w2 if kprod == 2 else t_w3
                ncols = 132 if kprod == 2 else 68
                psD = dn_pool.tile([128, 132], F32, tag="dn", name="dn")
                nc.tensor.matmul(psD[:, 0:ncols], xTt[:], wcat[:, 0:ncols],
                                 start=True, stop=True)
                epilogue_dense(kprod, w, psD)

            # ================= Layer 1 dense =================
            for w in range(NW):
                psD = dn_pool.tile([128, 132], F32, tag="dn", name="dn")
                nc.tensor.matmul(psD[:], t_xT[:, w * 128:(w + 1) * 128],
                                 t_w1[:], start=True, stop=True)
                epilogue_dense(1, w, psD)
            finalize_dense(1)

            # ================= 3 GAT layers =================
            qrr = [0]  # gather queue round-robin

            for k in (1, 2, 3)[:_nlayers]:
                hdim, elcol = EDGE[k]
                rec_full = rec_fulls[k]
                nc.gpsimd.collective_compute(
                    "AllGather", AL.bypass,
                    replica_groups=[list(range(N_CORES))],
                    ins=[rec_slices[k].opt()], outs=[rec_full.opt()],
                )
                for mb in blocks[:_nblocks]:
                    nch = mb["nch"]
                    nlo = mb["nlo_ch"] * 128
                    nhi = mb["nhi_ch"] * 128
                    # --- gather-independent work first: EQ, rep, STr, er ---
                    EQt = eqp.tile([128, CH, 128], BF, tag="EQ", name="EQ")
                    nc.vector.tensor_tensor(
                        EQt[:, 0:nch, :],
                        t_iota_row[:, None, :].to_broadcast((128, nch, 128)),
                        t_dstch[:, mb["ch_off"]:mb["ch_off"] + nch][:, :, None]
                        .to_broadcast((128, nch, 128)),
                        AL.is_equal)
                    rep = wp.tile([128, CH, 128], I8, tag="rep", name="rep")
                    nc.sync.dma_start(
                        out=rep[:, 0:nch, :],
                        in_=dstrow_d[0:1, mb["slot_off"]: mb["slot_off"] + nch * 128]
                        .to_broadcast((128, nch * 128)))
                    STrt = eqp.tile([128, CH, 128], BF, tag="STr", name="STr")
                    nc.vector.tensor_tensor(
                        STrt[:, 0:nch, :], rep[:, 0:nch, :],
                        t_iota8[:, :, None].to_broadcast((128, nch, 128)),
                        AL.is_equal)
                    ps_er = er_pool.tile([128, 2 * CH], F32, tag="er", name="er")
                    for ci in range(nch):
                        nc.tensor.matmul(
                            ps_er[:, 2 * ci:2 * ci + 2], STrt[:, ci, :],
                            t_er[k][:, mb["ch_w"][ci], :],
                            start=True, stop=True)
                    er2 = wp.tile([128, 2 * CH], F32, tag="er2", name="er2")
                    nc.vector.tensor_copy(er2[:, 0:2 * nch], ps_er[:, 0:2 * nch])
                    erc = wp.tile([128, CH], F32, tag="erc", name="erc")
                    nc.vector.tensor_tensor(
                        erc[:, 0:nch], er2[:, 0:2 * nch:2], er2[:, 1:2 * nch:2],
                        AL.add)
                    # --- gathers ---
                    rec = wp.tile([128, CH, REC], BF, tag="rec", name="rec")
                    for g0 in range(0, nlo, GCAP):
                        gn = min(GCAP, nlo - g0)
                        nc.gpsimd.dma_gather(
                            rec[:, g0 // 128:(g0 + gn) // 128, :],
                            rec_full[0:HI_BASE, :],
                            t_idxlo[:, mb["lo_col_off"] + g0 // 16:
                                    mb["lo_col_off"] + (g0 + gn) // 16],
                            gn, gn, REC, queue_num=qrr[0] % 4)
                        qrr[0] += 1
                    for g0 in range(0, nhi, GCAP):
                        gn = min(GCAP, nhi - g0)
                        nc.gpsimd.dma_gather(
                            rec[:, mb["nlo_ch"] + g0 // 128:
                                mb["nlo_ch"] + (g0 + gn) // 128, :],
                            rec_full[HI_BASE:NROWS, :],
                            t_idxhi[:, mb["hi_col_off"] + g0 // 16:
                                    mb["hi_col_off"] + (g0 + gn) // 16],
                            gn, gn, REC, queue_num=qrr[0] % 4)
                        qrr[0] += 1
                    # el + scores + weights (compact)
                    elc = wp.tile([128, CH], F32, tag="elc", name="elc")
                    nc.vector.tensor_copy(
                        elc[:, 0:nch],
                        rec[:, 0:nch, elcol:elcol + 2].bitcast(F32))
                    ec = wp.tile([128, CH], F32, tag="ec", name="ec")
                    nc.vector.tensor_tensor(ec[:, 0:nch], elc[:, 0:nch],
                                            erc[:, 0:nch], AL.add)
                    ex1 = wp.tile([128, CH], BF, tag="ex1", name="ex1")
                    nc.scalar.activation(ex1[:, 0:nch], ec[:, 0:nch],
                                         mybir.ActivationFunctionType.Exp,
                                         bias=0.0, scale=1.0)
                    ex2 = wp.tile([128, CH], BF, tag="ex2", name="ex2")
                    nc.scalar.activation(ex2[:, 0:nch], ec[:, 0:nch],
                                         mybir.ActivationFunctionType.Exp,
                                         bias=0.0, scale=0.2)
                    exc = wp.tile([128, CH], BF, tag="exc", name="exc")
                    nc.vector.tensor_tensor(exc[:, 0:nch], ex1[:, 0:nch],
                                            ex2[:, 0:nch], AL.max)
                    # scale gathered records (h + ones cols) in place by w_e
                    nc.vector.tensor_tensor(
                        rec[:, 0:nch, 0:hdim + 1], rec[:, 0:nch, 0:hdim + 1],
                        exc[:, 0:nch, None].to_broadcast((128, nch, hdim + 1)),
                        AL.mult)
                    # aggregation matmuls
                    psA = {w: agg_pool.tile([128, 132], F32, tag="agg",
                                            name=f"agg{w}")
                           for w in mb["wins"]}
                    for ci in range(nch):
                        w = mb["ch_w"][ci]
                        nc.tensor.matmul(
                            psA[w][:, 0:hdim + 1], EQt[:, ci, :],
                            rec[:, ci, 0:hdim + 1],
                            start=(ci == mb["first"][w]),
                            stop=(ci == mb["last"][w]))
                    # window epilogues
                    for w in mb["wins"]:
                        s_eps = wp.tile([128, 1], F32, tag="s_eps", name="s_eps")
                        nc.vector.tensor_scalar(
                            s_eps[:], psA[w][:, hdim:hdim + 1], 1e-30, None,
                            AL.add)
                        rcpv = wp.tile([128, 1], F32, tag="rcp", name="rcp")
                        nc.vector.reciprocal(rcpv[:], s_eps[:])
                        if k < 3:
                            xq = rcp.tile([128, 128], BF, tag="xq", name="xq")
                            nc.scalar.activation(
                                xq[:], psA[w][:, 0:hdim],
                                mybir.ActivationFunctionType.Relu,
                                bias=0.0, scale=rcpv[:])
                            dense_from_x(k + 1, w, xq[:])
                        else:
                            t3 = wp.tile([128, OUT_DIM], F32, tag="t3", name="t3")
                            nc.scalar.activation(
                                t3[:], psA[w][:, 0:OUT_DIM],
                                mybir.ActivationFunctionType.Relu,
                                bias=0.0, scale=rcpv[:])
                            nc.vector.tensor_tensor(
                                t_acc[:], t_acc[:], t3[:], AL.add)
                if k < 3:
                    finalize_dense(k + 1)

            # ================= readout =================
            psR = dn_pool.tile([128, 132], F32, tag="dn", name="dnR")
            nc.tensor.matmul(psR[0:1, 0:OUT_DIM], t_ones_col[:], t_acc[:],
                             start=True, stop=True)
            t_out = wp.tile([1, OUT_DIM], F32, tag="outv", name="outv")
            nc.vector.tensor_copy(t_out[:], psR[0:1, 0:OUT_DIM])
            nc.sync.dma_start(out=out_d[:], in_=t_out[:])

    nc.compile()
    return nc


def _hilo(v):
    hi = v.astype(BF16)
    lo = (v - hi.astype(np.float32)).astype(BF16)
    return hi, lo


def kernel(**inputs):
    global LAST_RESULTS
    x = np.ascontiguousarray(np.asarray(inputs["x"], np.float32))
    src = np.asarray(inputs["src"])
    dst = np.asarray(inputs["dst"])
    W1 = np.asarray(inputs["W1"], np.float32)
    al1 = np.asarray(inputs["al1"], np.float32)
    ar1 = np.asarray(inputs["ar1"], np.float32)
    W2 = np.asarray(inputs["W2"], np.float32)
    al2 = np.asarray(inputs["al2"], np.float32)
    ar2 = np.asarray(inputs["ar2"], np.float32)
    W3 = np.asarray(inputs["W3"], np.float32)
    al3 = np.asarray(inputs["al3"], np.float32)
    ar3 = np.asarray(inputs["ar3"], np.float32)

    key = hashlib.sha1(src.tobytes() + dst.tobytes()).hexdigest()
    if key not in _CACHE:
        t0 = time.time()
        meta = _host_prep(src, dst)
        t1 = time.time()
        nc = _build_program(meta)
        print(f"[kernel] host prep {t1 - t0:.1f}s, build+compile "
              f"{time.time() - t1:.1f}s", file=sys.stderr)
        _CACHE[key] = (meta, nc)
    meta, nc = _CACHE[key]

    w1ext = np.zeros((128, 132), np.float32)
    w1ext[:, 0:128] = W1
    w1ext[:, 128] = W1 @ al1
    w1ext[:, 129] = W1 @ ar1
    w2cat = np.zeros((128, 132), BF16)
    w2cat[:, 0:128] = W2.astype(BF16)
    w2cat[:, 128], w2cat[:, 129] = _hilo(W2 @ al2)
    w2cat[:, 130], w2cat[:, 131] = _hilo(W2 @ ar2)
    w3cat = np.zeros((128, 68), BF16)
    w3cat[:, 0:64] = W3.astype(BF16)
    w3cat[:, 64], w3cat[:, 65] = _hilo(W3 @ al3)
    w3cat[:, 66], w3cat[:, 67] = _hilo(W3 @ ar3)
    iota_row = np.tile(np.arange(128, dtype=np.float32).astype(BF16)[None, :],
                       (128, 1))
    iota_col8 = np.arange(128, dtype=np.int8).reshape(128, 1)
    eye = np.eye(128, dtype=np.float32).astype(BF16)

    in_maps = []
    for c in range(N_CORES):
        xs = np.zeros((SLICE_PAD, 128), np.float32)
        xs[0:SLICE] = x[c * SLICE:(c + 1) * SLICE]
        pc = meta["per_core"][c]
        in_maps.append(dict(
            xT=np.ascontiguousarray(xs.T),
            w1ext=w1ext, w2cat=w2cat, w3cat=w3cat,
            idxlo=pc["idx_lo"], idxhi=pc["idx_hi"],
            dstch=pc["dstch"], dstrow=pc["dstrow"],
            iota_row=iota_row, iota_col8=iota_col8, eye=eye,
        ))

    res = run_bass_kernel_spmd(nc, in_maps, core_ids=list(range(N_CORES)))
    LAST_RESULTS = res
    out = np.zeros(OUT_DIM, np.float32)
    for c in range(N_CORES):
        out += np.asarray(res.results[c]["out"], np.float32)[0]
    return out


# revision 16
# speedup vs baseline: 1.0778x; 1.0606x over previous
"""3-layer GAT on 8 Trainium2 NeuronCores (Bass/Tile).

Strategy (dst-sharded, edge phase fully local per core):
- Node dim sharded 8-ways: core c owns dst nodes [c*6250, (c+1)*6250).
- Host sorts edges by dst, groups them into 128-dst "windows" (49/core),
  splits each window's edges by record-row < 32768 (int16 gather index limit),
  and pads each (window, lo/hi) group to a multiple of 128 "slots". Group
  sizes are equalized across cores (max over cores) so one SPMD program
  serves all 8 cores.
- Per layer: a dense phase computes per-node records
  [h bf16 | 1.0 | el f32] (512B each) for the core's slice; an AllGather
  replicates the full 50176-row record table; the edge phase dma_gathers
  records by src (<=1024 rows per call, round-robin over 4 SWDGE queues),
  computes scores e = el[src] + er[dst] (er via one-hot-transpose matmuls
  against per-dst er hi/lo), weights w = exp(leaky_relu(e)) =
  max(exp(e), exp(0.2 e)), scales the gathered records in place by w, and
  aggregates out[d] = sum_e w_e*h[src_e] / sum_e w_e with one-hot EQ
  matmuls on the tensor engine (the record's ones-column supplies the
  denominator). All one-hot matrices are built with block-batched
  tensor_tensor compares on broadcast access patterns (no per-chunk
  AP-scalar tensor_scalar, which is ~10x slower).
- Layer k's edge-phase epilogue computes layer k+1's dense phase in place
  (ACT relu-with-reciprocal-scale, PE transpose, matmul with
  [W | wl_hi | wl_lo | wr_hi | wr_lo]); per-window el/er bookkeeping is
  staged and finalized with per-layer batched ops.
- Final readout: per-core partial sum over owned dst rows, host adds the 8
  partials.
"""
import hashlib
import os
import sys
import time

import numpy as np

sys.path.insert(0, "/opt/trn_rl_repo")
sys.path.insert(0, "/root/.axon_site/_ro/trn_rl_repo")

import ml_dtypes

import concourse.bacc as bacc
import concourse.mybir as mybir
import concourse.tile as tile
from concourse.bass_utils import run_bass_kernel_spmd
from concourse.library_config import mlp as mlp_lib

BF16 = ml_dtypes.bfloat16
F32 = mybir.dt.float32
BF = mybir.dt.bfloat16
I16 = mybir.dt.int16
I8 = mybir.dt.int8
AL = mybir.AluOpType

N_NODES = 50000
N_EDGES = 800000
HID = 128
OUT_DIM = 64
N_CORES = 8
SLICE = N_NODES // N_CORES          # 6250
NW = (SLICE + 127) // 128           # 49 windows per core
SLICE_PAD = NW * 128                # 6272
NROWS = SLICE_PAD * N_CORES         # 50176 record rows
HI_BASE = 32768                     # record rows >= HI_BASE use the offset gather
WPB = 2                             # windows per block
REC = 256                           # record bf16 elems (512B)
GCAP = 1024                         # max idxs per dma_gather call (SWDGE ring)

_CACHE = {}
LAST_RESULTS = None  # set by kernel(); test harness reads exec_time_ns off this


def _row_of_node(n):
    return (n // SLICE) * SLICE_PAD + (n % SLICE)


def _wrap_idxs(idx):
    """[n] int array -> [128, n/16] int16 wrapped-in-16, replicated across cores."""
    n = idx.shape[0]
    assert n % 16 == 0
    w = idx.reshape(n // 16, 16).T
    return np.tile(w, (8, 1)).astype(np.int16)


def _host_prep(src, dst):
    src = np.asarray(src).astype(np.int64)
    dst = np.asarray(dst).astype(np.int64)
    rows = _row_of_node(src)

    per_cw = {}
    n_lo = np.zeros((N_CORES, NW), np.int64)
    n_hi = np.zeros((N_CORES, NW), np.int64)
    for c in range(N_CORES):
        m = (dst >= c * SLICE) & (dst < (c + 1) * SLICE)
        dl = dst[m] - c * SLICE
        rs = rows[m]
        order = np.argsort(dl, kind="stable")
        dl = dl[order]
        rs = rs[order]
        bounds = np.searchsorted(dl, np.arange(0, SLICE_PAD + 1, 128))
        for w in range(NW):
            a, b = bounds[w], bounds[w + 1]
            wdl = dl[a:b]
            wrs = rs[a:b]
            lo = wrs < HI_BASE
            per_cw[(c, w)] = (wdl[lo], wrs[lo], wdl[~lo], wrs[~lo])
            n_lo[c, w] = lo.sum()
            n_hi[c, w] = (~lo).sum()

    L = np.maximum(1, -(-n_lo.max(axis=0) // 128))
    H = np.maximum(1, -(-n_hi.max(axis=0) // 128))

    blocks = [list(range(b, min(b + WPB, NW))) for b in range(0, NW, WPB)]

    meta_blocks = []
    tot_ch = 0
    tot_slots = 0
    lo_cols = 0
    hi_cols = 0
    for wins in blocks:
        nlo_ch = int(sum(L[w] for w in wins))
        nhi_ch = int(sum(H[w] for w in wins))
        nch = nlo_ch + nhi_ch
        ch_w = []
        for w in wins:
            ch_w += [w] * int(L[w])
        for w in wins:
            ch_w += [w] * int(H[w])
        first = {w: ch_w.index(w) for w in wins}
        last = {w: len(ch_w) - 1 - ch_w[::-1].index(w) for w in wins}
        meta_blocks.append(dict(
            wins=wins, nlo_ch=nlo_ch, nhi_ch=nhi_ch, nch=nch, ch_w=ch_w,
            first=first, last=last,
            ch_off=tot_ch, slot_off=tot_slots,
            lo_col_off=lo_cols, hi_col_off=hi_cols,
        ))
        tot_ch += nch
        tot_slots += nch * 128
        lo_cols += nlo_ch * 128 // 16
        hi_cols += nhi_ch * 128 // 16

    per_core = []
    for c in range(N_CORES):
        idx_lo_cols = []
        idx_hi_cols = []
        dstch = np.full((128, tot_ch), -1.0, np.float32)
        dstrow = np.full((1, tot_slots), -1, np.int8)
        for mb in meta_blocks:
            lo_idx = []
            hi_idx = []
            lo_dl = []
            hi_dl = []
            for w in mb["wins"]:
                wdl_lo, wrs_lo, wdl_hi, wrs_hi = per_cw[(c, w)]
                npad = int(L[w]) * 128
                pad_i = np.zeros(npad, np.int64)
                pad_d = np.full(npad, -1.0, np.float64)
                pad_i[: len(wrs_lo)] = wrs_lo
                pad_d[: len(wdl_lo)] = wdl_lo - w * 128
                lo_idx.append(pad_i)
                lo_dl.append(pad_d)
                npad = int(H[w]) * 128
                pad_i = np.zeros(npad, np.int64)
                pad_d = np.full(npad, -1.0, np.float64)
                pad_i[: len(wrs_hi)] = wrs_hi - HI_BASE
                pad_d[: len(wdl_hi)] = wdl_hi - w * 128
                hi_idx.append(pad_i)
                hi_dl.append(pad_d)
            lo_idx = np.concatenate(lo_idx)
            hi_idx = np.concatenate(hi_idx)
            dl_all = np.concatenate(lo_dl + hi_dl)
            idx_lo_cols.append(_wrap_idxs(lo_idx))
            idx_hi_cols.append(_wrap_idxs(hi_idx))
            nch = mb["nch"]
            dstch[:, mb["ch_off"]: mb["ch_off"] + nch] = (
                dl_all.reshape(nch, 128).T.astype(np.float32))
            dstrow[0, mb["slot_off"]: mb["slot_off"] + nch * 128] = (
                dl_all.astype(np.int8))
        per_core.append(dict(
            idx_lo=np.concatenate(idx_lo_cols, axis=1),
            idx_hi=np.concatenate(idx_hi_cols, axis=1),
            dstch=dstch.astype(BF16),
            dstrow=dstrow,
        ))

    return dict(
        blocks=meta_blocks, L=L, H=H, per_core=per_core,
        tot_ch=tot_ch, tot_slots=tot_slots,
        lo_cols=lo_cols, hi_cols=hi_cols,
        chmax=max(mb["nch"] for mb in meta_blocks),
    )


def _build_program(meta):
    nc = bacc.Bacc("TRN2", target_bir_lowering=False, debug=False,
                   num_devices=N_CORES, num_swdge_queues=4)

    xT = nc.dram_tensor("xT", [128, SLICE_PAD], F32, kind="ExternalInput")
    w1ext = nc.dram_tensor("w1ext", [128, 132], F32, kind="ExternalInput")
    w2cat = nc.dram_tensor("w2cat", [128, 132], BF, kind="ExternalInput")
    w3cat = nc.dram_tensor("w3cat", [128, 68], BF, kind="ExternalInput")
    idxlo_d = nc.dram_tensor("idxlo", [128, meta["lo_cols"]], I16, kind="ExternalInput")
    idxhi_d = nc.dram_tensor("idxhi", [128, meta["hi_cols"]], I16, kind="ExternalInput")
    dstch_d = nc.dram_tensor("dstch", [128, meta["tot_ch"]], BF, kind="ExternalInput")
    dstrow_d = nc.dram_tensor("dstrow", [1, meta["tot_slots"]], I8, kind="ExternalInput")
    iota_row_d = nc.dram_tensor("iota_row", [128, 128], BF, kind="ExternalInput")
    iota_col8_d = nc.dram_tensor("iota_col8", [128, 1], I8, kind="ExternalInput")
    eye_d = nc.dram_tensor("eye", [128, 128], BF, kind="ExternalInput")
    out_d = nc.dram_tensor("out", [1, OUT_DIM], F32, kind="ExternalOutput")

    # edge-phase params per layer: (h dim in record, el bf16-col)
    EDGE = {1: (HID, 130), 2: (HID, 130), 3: (OUT_DIM, 66)}
    blocks = meta["blocks"]
    CH = meta["chmax"]

    _nlayers = int(os.environ.get("GAT_LAYERS", "3"))
    _nblocks = int(os.environ.get("GAT_BLOCKS", "999"))

    with tile.TileContext(nc) as tc:
        with (
            tc.tile_pool(name="resid", bufs=1) as rp,
            tc.tile_pool(name="work", bufs=2) as wp,
            tc.tile_pool(name="eqp", bufs=2) as eqp,
            tc.tile_pool(name="recs", bufs=3) as rcp,
            tc.tile_pool(name="dram", bufs=1, space="DRAM") as dp,
            tc.tile_pool(name="agg_ps", bufs=4, space="PSUM") as agg_pool,
            tc.tile_pool(name="er_ps", bufs=1, space="PSUM") as er_pool,
            tc.tile_pool(name="dn_ps", bufs=1, space="PSUM") as dn_pool,
            tc.tile_pool(name="tr_ps", bufs=1, space="PSUM") as tr_pool,
        ):
            nc.gpsimd.load_library(mlp_lib)

            # ---- resident tiles ----
            t_xT = rp.tile([128, SLICE_PAD], F32, tag="xT")
            nc.sync.dma_start(out=t_xT[:], in_=xT[:])
            t_w1 = rp.tile([128, 132], F32, tag="w1")
            nc.sync.dma_start(out=t_w1[:], in_=w1ext[:])
            t_w2 = rp.tile([128, 132], BF, tag="w2")
            nc.sync.dma_start(out=t_w2[:], in_=w2cat[:])
            t_w3 = rp.tile([128, 68], BF, tag="w3")
            nc.sync.dma_start(out=t_w3[:], in_=w3cat[:])
            t_idxlo = rp.tile([128, meta["lo_cols"]], I16, tag="idxlo")
            nc.sync.dma_start(out=t_idxlo[:], in_=idxlo_d[:])
            t_idxhi = rp.tile([128, meta["hi_cols"]], I16, tag="idxhi")
            nc.sync.dma_start(out=t_idxhi[:], in_=idxhi_d[:])
            t_dstch = rp.tile([128, meta["tot_ch"]], BF, tag="dstch")
            nc.sync.dma_start(out=t_dstch[:], in_=dstch_d[:])
            t_iota_row = rp.tile([128, 128], BF, tag="iota_row")
            nc.sync.dma_start(out=t_iota_row[:], in_=iota_row_d[:])
            t_iota8 = rp.tile([128, 1], I8, tag="iota8")
            nc.sync.dma_start(out=t_iota8[:], in_=iota_col8_d[:])
            t_eye = rp.tile([128, 128], BF, tag="eye")
            nc.sync.dma_start(out=t_eye[:], in_=eye_d[:])

            t_er = {k: rp.tile([128, NW, 2], BF, tag=f"er{k}", name=f"er{k}")
                    for k in (1, 2, 3)}
            # per-produced-layer staging of [el(2) | er(2)] psD cols
            t_stage = {k: rp.tile([128, NW, 4], F32, tag=f"st{k}", name=f"st{k}")
                       for k in (1, 2, 3)}
            t_acc = rp.tile([128, OUT_DIM], F32, tag="acc")
            nc.vector.memset(t_acc[:], 0.0)
            t_ones_col = rp.tile([128, 1], F32, tag="ones_col")
            nc.vector.memset(t_ones_col[:], 1.0)
            ones_bf = nc.const_aps.tensor(1.0, (128, 1), BF)

            rec_slices = {k: dp.tile([SLICE_PAD, REC], BF, name=f"rec_slice{k}")
                          for k in (1, 2, 3)}
            rec_fulls = {k: dp.tile([NROWS, REC], BF, addr_space="Shared",
                                    name=f"rec_full{k}")
                         for k in (1, 2, 3)}

            def epilogue_dense(kprod, w, psD):
                """Per-window: record h+ones assembly + stage el/er cols."""
                hdim = HID if kprod < 3 else OUT_DIM
                rec_t = rcp.tile([128, REC], BF, tag="rec_out", name="rec_out")
                nc.vector.memset(rec_t[:], 0.0)
                nc.scalar.copy(rec_t[:, 0:hdim], psD[:, 0:hdim])
                nc.vector.tensor_copy(rec_t[:, hdim:hdim + 1], ones_bf)
                if kprod == 1:
                    nc.scalar.copy(t_stage[1][:, w, 0:2], psD[:, 128:130])
                else:
                    nc.scalar.copy(t_stage[kprod][:, w, :], psD[:, hdim:hdim + 4])
                nc.sync.dma_start(
                    out=rec_slices[kprod][w * 128:(w + 1) * 128, :], in_=rec_t[:])

            def finalize_dense(kprod):
                """Per-layer batched: el -> record table, er -> hi/lo."""
                hdim = HID if kprod < 3 else OUT_DIM
                elcol = hdim + 2
                st = t_stage[kprod]
                elv = wp.tile([128, NW], F32, tag="elv", name="elv")
                erv = wp.tile([128, NW], F32, tag="erv", name="erv")
                if kprod == 1:
                    nc.vector.tensor_copy(elv[:], st[:, :, 0])
                    nc.vector.tensor_copy(erv[:], st[:, :, 1])
                else:
                    nc.vector.tensor_tensor(elv[:], st[:, :, 0], st[:, :, 1], AL.add)
                    nc.vector.tensor_tensor(erv[:], st[:, :, 2], st[:, :, 3], AL.add)
                # el into the record table's el field (strided 4B writes)
                el_dst = (rec_slices[kprod]
                          .rearrange("(w p) j -> p w j", p=128)[:, :, elcol:elcol + 2]
                          .bitcast(F32))
                nc.sync.dma_start(out=el_dst, in_=elv[:, :, None])
                # er hi/lo
                erh = t_er[kprod][:, :, 0:1]
                erl = t_er[kprod][:, :, 1:2]
                nc.vector.tensor_copy(erh, erv[:, :, None])
                t32 = wp.tile([128, NW], F32, tag="er32", name="er32")
                nc.vector.tensor_copy(t32[:], erh)
                nc.vector.tensor_tensor(t32[:], erv[:], t32[:], AL.subtract)
                nc.vector.tensor_copy(erl, t32[:, :, None])

            def dense_from_x(kprod, w, xq_bf):
                ps_t = tr_pool.tile([128, 128], BF, tag="tr", name="tr")
                nc.tensor.transpose(ps_t[:], xq_bf, t_eye[:])
                xTt = wp.tile([128, 128], BF, tag="xTt", name="xTt")
                nc.scalar.copy(xTt[:], ps_t[:])
                wcat = t_w2 if kprod == 2 else t_w3
                ncols = 132 if kprod == 2 else 68
                psD = dn_pool.tile([128, 132], F32, tag="dn", name="dn")
                nc.tensor.matmul(psD[:, 0:ncols], xTt[:], wcat[:, 0:ncols],
                                 start=True, stop=True)
                epilogue_dense(kprod, w, psD)

            # ================= Layer 1 dense =================
            for w in range(NW):
                psD = dn_pool.tile([128, 132], F32, tag="dn", name="dn")
                nc.tensor.matmul(psD[:], t_xT[:, w * 128:(w + 1) * 128],
                                 t_w1[:], start=True, stop=True)
                epilogue_dense(1, w, psD)
            finalize_dense(1)

            # ================= 3 GAT layers =================
            qrr = [0]  # gather queue round-robin

            for k in (1, 2, 3)[:_nlayers]:
                hdim, elcol = EDGE[k]
                rec_full = rec_fulls[k]
                nc.gpsimd.collective_compute(
                    "AllGather", AL.bypass,
                    replica_groups=[list(range(N_CORES))],
                    ins=[rec_slices[k].opt()], outs=[rec_full.opt()],
                )
                for mb in blocks[:_nblocks]:
                    nch = mb["nch"]
                    nlo = mb["nlo_ch"] * 128
                    nhi = mb["nhi_ch"] * 128
                    # --- gather-independent work first: EQ, rep, STr, er ---
                    EQt = eqp.tile([128, CH, 128], BF, tag="EQ", name="EQ")
                    nc.vector.tensor_tensor(
                        EQt[:, 0:nch, :],
                        t_iota_row[:, None, :].to_broadcast((128, nch, 128)),
                        t_dstch[:, mb["ch_off"]:mb["ch_off"] + nch][:, :, None]
                        .to_broadcast((128, nch, 128)),
                        AL.is_equal)
                    rep = wp.tile([128, CH, 128], I8, tag="rep", name="rep")
                    nc.sync.dma_start(
                        out=rep[:, 0:nch, :],
                        in_=dstrow_d[0:1, mb["slot_off"]: mb["slot_off"] + nch * 128]
                        .to_broadcast((128, nch * 128)))
                    STrt = eqp.tile([128, CH, 128], BF, tag="STr", name="STr")
                    nc.vector.tensor_tensor(
                        STrt[:, 0:nch, :], rep[:, 0:nch, :],
                        t_iota8[:, :, None].to_broadcast((128, nch, 128)),
                        AL.is_equal)
                    ps_er = er_pool.tile([128, 2 * CH], F32, tag="er", name="er")
                    for ci in range(nch):
                        nc.tensor.matmul(
                            ps_er[:, 2 * ci:2 * ci + 2], STrt[:, ci, :],
                            t_er[k][:, mb["ch_w"][ci], :],
                            start=True, stop=True)
                    er2 = wp.tile([128, 2 * CH], F32, tag="er2", name="er2")
                    nc.vector.tensor_copy(er2[:, 0:2 * nch], ps_er[:, 0:2 * nch])
                    erc = wp.tile([128, CH], F32, tag="erc", name="erc")
                    nc.vector.tensor_tensor(
                        erc[:, 0:nch], er2[:, 0:2 * nch:2], er2[:, 1:2 * nch:2],
                        AL.add)
                    # --- gathers ---
                    rec = wp.tile([128, CH, REC], BF, tag="rec", name="rec", bufs=3)
                    for g0 in range(0, nlo, GCAP):
                        gn = min(GCAP, nlo - g0)
                        nc.gpsimd.dma_gather(
                            rec[:, g0 // 128:(g0 + gn) // 128, :],
                            rec_full[0:HI_BASE, :],
                            t_idxlo[:, mb["lo_col_off"] + g0 // 16:
                                    mb["lo_col_off"] + (g0 + gn) // 16],
                            gn, gn, REC, queue_num=qrr[0] % 4)
                        qrr[0] += 1
                    for g0 in range(0, nhi, GCAP):
                        gn = min(GCAP, nhi - g0)
                        nc.gpsimd.dma_gather(
                            rec[:, mb["nlo_ch"] + g0 // 128:
                                mb["nlo_ch"] + (g0 + gn) // 128, :],
                            rec_full[HI_BASE:NROWS, :],
                            t_idxhi[:, mb["hi_col_off"] + g0 // 16:
                                    mb["hi_col_off"] + (g0 + gn) // 16],
                            gn, gn, REC, queue_num=qrr[0] % 4)
                        qrr[0] += 1
                    # el + scores + weights (compact)
                    elc = wp.tile([128, CH], F32, tag="elc", name="elc")
                    nc.vector.tensor_copy(
                        elc[:, 0:nch],
                        rec[:, 0:nch, elcol:elcol + 2].bitcast(F32))
                    ec = wp.tile([128, CH], F32, tag="ec", name="ec")
                    nc.vector.tensor_tensor(ec[:, 0:nch], elc[:, 0:nch],
                                            erc[:, 0:nch], AL.add)
                    ex1 = wp.tile([128, CH], BF, tag="ex1", name="ex1")
                    nc.scalar.activation(ex1[:, 0:nch], ec[:, 0:nch],
                                         mybir.ActivationFunctionType.Exp,
                                         bias=0.0, scale=1.0)
                    ex2 = wp.tile([128, CH], BF, tag="ex2", name="ex2")
                    nc.scalar.activation(ex2[:, 0:nch], ec[:, 0:nch],
                                         mybir.ActivationFunctionType.Exp,
                                         bias=0.0, scale=0.2)
                    exc = wp.tile([128, CH], BF, tag="exc", name="exc")
                    nc.vector.tensor_tensor(exc[:, 0:nch], ex1[:, 0:nch],
                                            ex2[:, 0:nch], AL.max)
                    # scale gathered records (h + ones cols) in place by w_e
                    nc.vector.tensor_tensor(
                        rec[:, 0:nch, 0:hdim + 1], rec[:, 0:nch, 0:hdim + 1],
                        exc[:, 0:nch, None].to_broadcast((128, nch, hdim + 1)),
                        AL.mult)
                    # aggregation matmuls
                    psA = {w: agg_pool.tile([128, 132], F32, tag="agg",
                                            name=f"agg{w}")
                           for w in mb["wins"]}
                    for ci in range(nch):
                        w = mb["ch_w"][ci]
                        nc.tensor.matmul(
                            psA[w][:, 0:hdim + 1], EQt[:, ci, :],
                            rec[:, ci, 0:hdim + 1],
                            start=(ci == mb["first"][w]),
                            stop=(ci == mb["last"][w]))
                    # window epilogues
                    for w in mb["wins"]:
                        s_eps = wp.tile([128, 1], F32, tag="s_eps", name="s_eps")
                        nc.vector.tensor_scalar(
                            s_eps[:], psA[w][:, hdim:hdim + 1], 1e-30, None,
                            AL.add)
                        rcpv = wp.tile([128, 1], F32, tag="rcp", name="rcp")
                        nc.vector.reciprocal(rcpv[:], s_eps[:])
                        if k < 3:
                            xq = rcp.tile([128, 128], BF, tag="xq", name="xq")
                            nc.scalar.activation(
                                xq[:], psA[w][:, 0:hdim],
                                mybir.ActivationFunctionType.Relu,
                                bias=0.0, scale=rcpv[:])
                            dense_from_x(k + 1, w, xq[:])
                        else:
                            t3 = wp.tile([128, OUT_DIM], F32, tag="t3", name="t3")
                            nc.scalar.activation(
                                t3[:], psA[w][:, 0:OUT_DIM],
                                mybir.ActivationFunctionType.Relu,
                                bias=0.0, scale=rcpv[:])
                            nc.vector.tensor_tensor(
                                t_acc[:], t_acc[:], t3[:], AL.add)
                if k < 3:
                    finalize_dense(k + 1)

            # ================= readout =================
            psR = dn_pool.tile([128, 132], F32, tag="dn", name="dnR")
            nc.tensor.matmul(psR[0:1, 0:OUT_DIM], t_ones_col[:], t_acc[:],
                             start=True, stop=True)
            t_out = wp.tile([1, OUT_DIM], F32, tag="outv", name="outv")
            nc.vector.tensor_copy(t_out[:], psR[0:1, 0:OUT_DIM])
            nc.sync.dma_start(out=out_d[:], in_=t_out[:])

    nc.compile()
    return nc


def _hilo(v):
    hi = v.astype(BF16)
    lo = (v - hi.astype(np.float32)).astype(BF16)
    return hi, lo


def kernel(**inputs):
    global LAST_RESULTS
    x = np.ascontiguousarray(np.asarray(inputs["x"], np.float32))
    src = np.asarray(inputs["src"])
    dst = np.asarray(inputs["dst"])
    W1 = np.asarray(inputs["W1"], np.float32)
    al1 = np.asarray(inputs["al1"], np.float32)
    ar1 = np.asarray(inputs["ar1"], np.float32)
    W2 = np.asarray(inputs["W2"], np.float32)
    al2 = np.asarray(inputs["al2"], np.float32)
    ar2 = np.asarray(inputs["ar2"], np.float32)
    W3 = np.asarray(inputs["W3"], np.float32)
    al3 = np.asarray(inputs["al3"], np.float32)
    ar3 = np.asarray(inputs["ar3"], np.float32)

    key = hashlib.sha1(src.tobytes() + dst.tobytes()).hexdigest()
    if key not in _CACHE:
        t0 = time.time()
        meta = _host_prep(src, dst)
        t1 = time.time()
        nc = _build_program(meta)
        print(f"[kernel] host prep {t1 - t0:.1f}s, build+compile "
              f"{time.time() - t1:.1f}s", file=sys.stderr)
        _CACHE[key] = (meta, nc)
    meta, nc = _CACHE[key]

    w1ext = np.zeros((128, 132), np.float32)
    w1ext[:, 0:128] = W1
    w1ext[:, 128] = W1 @ al1
    w1ext[:, 129] = W1 @ ar1
    w2cat = np.zeros((128, 132), BF16)
    w2cat[:, 0:128] = W2.astype(BF16)
    w2cat[:, 128], w2cat[:, 129] = _hilo(W2 @ al2)
    w2cat[:, 130], w2cat[:, 131] = _hilo(W2 @ ar2)
    w3cat = np.zeros((128, 68), BF16)
    w3cat[:, 0:64] = W3.astype(BF16)
    w3cat[:, 64], w3cat[:, 65] = _hilo(W3 @ al3)
    w3cat[:, 66], w3cat[:, 67] = _hilo(W3 @ ar3)
    iota_row = np.tile(np.arange(128, dtype=np.float32).astype(BF16)[None, :],
                       (128, 1))
    iota_col8 = np.arange(128, dtype=np.int8).reshape(128, 1)
    eye = np.eye(128, dtype=np.float32).astype(BF16)

    in_maps = []
    for c in range(N_CORES):
        xs = np.zeros((SLICE_PAD, 128), np.float32)
        xs[0:SLICE] = x[c * SLICE:(c + 1) * SLICE]
        pc = meta["per_core"][c]
        in_maps.append(dict(
            xT=np.ascontiguousarray(xs.T),
            w1ext=w1ext, w2cat=w2cat, w3cat=w3cat,
            idxlo=pc["idx_lo"], idxhi=pc["idx_hi"],
            dstch=pc["dstch"], dstrow=pc["dstrow"],
            iota_row=iota_row, iota_col8=iota_col8, eye=eye,
        ))

    res = run_bass_kernel_spmd(nc, in_maps, core_ids=list(range(N_CORES)))
    LAST_RESULTS = res
    out = np.zeros(OUT_DIM, np.float32)
    for c in range(N_CORES):
        out += np.asarray(res.results[c]["out"], np.float32)[0]
    return out
